# revision 40
# baseline (speedup 1.0000x reference)
"""Trainium2 Bass kernel for the grouped contrastive loss (v2).

Math: for anchors i and positives j in the same sensitive-attribute
group g (size P),
    row(i,j) = S_ij - D * ln E_ij
with S_ij = <p_i, p_j>/t and E_ij = sum_d exp(p_i[d] p_j[d] / t)
(the log-softmax max-shift cancels analytically), and
    loss = sum_i -1/(N P_i^2) * sum_{j in g(i)} row(i,j).

v2 exploits the symmetry row(i,j) == row(j,i): sort points by group so
same-group pairs are dense blocks; for each (128-anchor block, group)
job the device computes one window = [own-block cols (padded to 128) |
suffix = all later same-group cols].  Within-block ordered pairs are
counted at weight 1 (both orders present in the own part); cross-block
unordered pairs appear once in the earlier block's suffix and get
weight 2.  Per slot the device produces
    Ssum[i]  = x_i . ybar_m          (ybar_m = sum_own y + 2*sum_sfx y)
    Lred[i]  = sum_{j in window} ln E_ij
    Lown[i]  = sum_{j in first 128 cols} ln E_ij
and the final per-row combine is
    acc = sum_s wS*Ssum + wA*Lred + wB*Lown + kwcol
with wA = -2*D*wS, wB = +D*wS (own part => net weight 1), and kwcol the
exact host-side correction for zero-padded dummy columns (E = D there).

Device pipeline per slot (W columns, 128 anchors as 32 packs x 4):
  DVE  : 32x tensor_scalar_mul bf16 -> prod [128, 32W]
  ACT  : 4x Exp chunks [128, 8W] (bf16), 1x Ln [128, W] PSUM->SBUF
         (Exp+Ln forced into the one 'natural_log_exp_and_others'
         table set -- no per-slot ACT_TABLE_LOADs)
  PE   : 32x block-diag matmuls accumulate exp rows -> E [128, W] PSUM,
         plus one 1-column fp32 matmul for Ssum
  DVE  : 2x tensor_reduce of ln E -> Lred/Lown columns
Final: 3 chained tensor_tensor_reduce -> acc [128,1]; host sums cores.
"""

import math
import os
import sys

sys.path.insert(0, "/opt/trn_rl_repo")

import numpy as np
import ml_dtypes

import concourse.bacc as bacc
import concourse.bass as bass
import concourse.tile as tile
from concourse import mybir
from concourse.bass_utils import run_bass_kernel_spmd

N_CORES = 8
D = 32
LN_D = math.log(float(D))
SPLIT = 384  # max window width (PSUM bank limit 512 fp32; 384 packs best)

last_run_info = {}


def _install_ntff_hook():
    # bass_utils' trace path under axon imports antenv.axon_hooks, which is
    # absent in this image; provide the ctypes-based hook it expects.
    import contextlib
    import ctypes
    import types

    if "antenv.axon_hooks" in sys.modules:
        return

    def _make_hook():
        try:
            lib = ctypes.CDLL("/opt/axon/libaxon_pjrt.so")
        except OSError:
            return None
        if not hasattr(lib, "axon_start_nrt_profile"):
            return None
        lib.axon_start_nrt_profile.argtypes = [
            ctypes.POINTER(ctypes.c_int64),
            ctypes.c_size_t,
        ]
        lib.axon_start_nrt_profile.restype = ctypes.c_int64
        lib.axon_stop_nrt_profile.argtypes = [ctypes.c_char_p]
        lib.axon_stop_nrt_profile.restype = ctypes.c_int64

        @contextlib.contextmanager
        def _hook_cm(output_dir, device_ids):
            import jax

            jax.devices()
            if device_ids:
                ids = (ctypes.c_int64 * len(device_ids))(*device_ids)
                rc = lib.axon_start_nrt_profile(ids, len(device_ids))
            else:
                rc = lib.axon_start_nrt_profile(None, 0)
            if rc != 0:
                raise RuntimeError(f"axon_start_nrt_profile rc={rc}")
            try:
                yield
            finally:
                n = lib.axon_stop_nrt_profile(str(output_dir).encode())
                if n < 0:
                    raise RuntimeError(f"axon_stop_nrt_profile rc={n}")

        return _hook_cm

    hook = _make_hook()
    mod = types.ModuleType("antenv.axon_hooks")
    mod.get_axon_ntff_profile_hook = lambda: hook
    mod.set_axon_ntff_profile_hook = lambda h: None
    sys.modules["antenv.axon_hooks"] = mod


def _install_act_table_patch():
    # Exp and Ln both live in the 'natural_log_exp_and_others' activation
    # table set; by default the table-load inserter resolves each function
    # to the first set containing it, so alternating Exp/Ln reloads tables
    # every transition (~1.3us each).  Strip Exp/Ln from every other set so
    # both resolve to the combined one -> a single hoisted load.
    if os.environ.get("ACT_TBL_PATCH", "1") != "1":
        return
    if getattr(bacc, "_act_tbl_patched", False):
        return
    orig = bacc.get_activation_tables
    Exp = mybir.ActivationFunctionType.Exp
    Ln = mybir.ActivationFunctionType.Ln

    def patched(arch):
        tabs = orig(arch)
        combined = "natural_log_exp_and_others"
        if combined not in tabs or not {Exp, Ln} <= tabs[combined]:
            return tabs
        return {
            name: (fns if name == combined else fns - {Exp, Ln})
            for name, fns in tabs.items()
        }

    bacc.get_activation_tables = patched
    bacc._act_tbl_patched = True


def _plan(sa_sorted):
    """Build symmetric slots and assign to cores.

    Slot = dict(pos0, lo, hi, P, own=(col0, L1) | None, sfx=(col0, ls), w).
    Rows of the slot are sorted positions [pos0+lo, pos0+hi); window
    layout: own cols at local [0, L1), zeros to 128, suffix at [128, ...)
    for own-slots; pure suffix at [0, ls) for tail slots.

    Returns (widths, per_core) with per_core[c] a list of len(widths)
    entries (slot dict or None), widths[s] the compile-time window width.
    """
    n = len(sa_sorted)
    assert n % 128 == 0
    bounds = [0]
    for i in range(1, n):
        if sa_sorted[i] != sa_sorted[i - 1]:
            bounds.append(i)
    bounds.append(n)
    groups = [(bounds[i], bounds[i + 1]) for i in range(len(bounds) - 1)]

    slots = []
    for b in range(n // 128):
        pos0 = 128 * b
        for g0, g1 in groups:
            lo, hi = max(pos0, g0), min(pos0 + 128, g1)
            if lo >= hi:
                continue
            P = g1 - g0
            L2 = max(0, g1 - (pos0 + 128))
            f = min(L2, SPLIT - 128)
            slots.append(
                dict(
                    pos0=pos0,
                    lo=lo - pos0,
                    hi=hi - pos0,
                    P=P,
                    own=(lo, hi - lo),
                    sfx=(pos0 + 128, f),
                    w=128 + f,
                )
            )
            c0, rem = pos0 + 128 + f, L2 - f
            while rem > 0:
                l = min(rem, SPLIT)
                slots.append(
                    dict(
                        pos0=pos0,
                        lo=lo - pos0,
                        hi=hi - pos0,
                        P=P,
                        own=None,
                        sfx=(c0, l),
                        w=l,
                    )
                )
                c0 += l
                rem -= l

    slots.sort(key=lambda s: -s["w"])
    ns = (len(slots) + N_CORES - 1) // N_CORES
    per_core = [[] for _ in range(N_CORES)]
    for k in range(ns):
        rank = slots[N_CORES * k : N_CORES * (k + 1)]
        order = range(N_CORES) if k % 2 == 0 else range(N_CORES - 1, -1, -1)
        it = iter(rank)
        assign = {}
        for c in order:
            assign[c] = next(it, None)
        for c in range(N_CORES):
            per_core[c].append(assign[c])
    widths = []
    for s in range(ns):
        wmax = max(p[s]["w"] if p[s] is not None else 0 for p in per_core)
        widths.append(max(32, int(math.ceil(wmax / 32.0)) * 32))
    # smallest slot first: cheapest pipeline fill (small first DMA, small
    # first prod batch) and the big slots stream while DMAs continue
    widths.reverse()
    for p in per_core:
        p.reverse()
    return widths, per_core


def _build_program(widths):
    nc = bacc.Bacc(
        "TRN2", target_bir_lowering=False, debug=False, num_devices=N_CORES
    )
    f32 = mybir.dt.float32
    bf16 = mybir.dt.bfloat16
    ns = len(widths)
    TW = sum(widths)
    Wmax = max(widths)
    offs = [sum(widths[:s]) for s in range(ns)]

    # bf16 uploads: per slot s one [128, W_s + 32] block (window | scal),
    # slot 0's block carries the 256-col onesbd appended.
    # f32 upload fmat: wS | wA | wB | kw | ysum (rows 0:32) | lhsa_s blocks
    # (rows 0:32), one DMA each so compute starts after slot 0's block.
    bw = [widths[s] + (256 if s == 0 else 0) for s in range(ns)]
    rx_d = [
        nc.dram_tensor(f"rx{s}", [128, bw[s]], bf16, kind="ExternalInput").ap()
        for s in range(ns)
    ]
    # scmat: per-slot per-pack anchor scalars (prods gate on this — small,
    # shipped first). fmat: weights/ysum/lhsa (needed later in each slot).
    scmat_d = nc.dram_tensor("scmat", [128, 32 * ns], f32, kind="ExternalInput").ap()
    FC = 3 * ns + 1 + ns + 128 * ns
    fmat_d = nc.dram_tensor("fmat", [128, FC], f32, kind="ExternalInput").ap()
    out_d = nc.dram_tensor("out", [128, 1], f32, kind="ExternalOutput").ap()

    Exp = mybir.ActivationFunctionType.Exp
    Ln = mybir.ActivationFunctionType.Ln

    with tile.TileContext(nc) as tc:
        with (
            tc.tile_pool(name="const", bufs=1) as cpool,
            tc.tile_pool(name="prod", bufs=2) as ppool,
            tc.tile_pool(name="expt", bufs=2) as epool,
            tc.tile_pool(name="red", bufs=2) as rpool,
            tc.tile_pool(name="psE", bufs=2, space="PSUM") as psE,
            tc.tile_pool(name="psS", bufs=1, space="PSUM") as psS,
        ):
            rxs = []
            for s in range(ns):
                rxs.append(
                    cpool.tile([128, bw[s]], bf16, tag=f"rx{s}", name=f"rx{s}")
                )
            scmat = cpool.tile([128, 32 * ns], f32, tag="scmat")
            fmat = cpool.tile([128, FC], f32, tag="fmat")
            wS = fmat[:, 0:ns]
            wA = fmat[:, ns : 2 * ns]
            wB = fmat[:, 2 * ns : 3 * ns]
            kw = fmat[:, 3 * ns : 3 * ns + 1]
            yc = 3 * ns + 1
            lc = yc + ns
            onesbd = rxs[0][:, widths[0] : widths[0] + 256]

            nc.sync.dma_start(scmat[:], scmat_d[:])
            nc.sync.dma_start(rxs[0][:], rx_d[0][:])
            for s in range(1, ns):
                nc.gpsimd.dma_start(rxs[s][:], rx_d[s][:])
            nc.gpsimd.dma_start(fmat[:], fmat_d[:])

            logE = cpool.tile([128, TW], f32, tag="logE")
            acc = rpool.tile([128, 1], f32, tag="acc")
            nc.vector.tensor_copy(acc[:], kw)
            Ssum = psS.tile([128, ns], f32, tag="Ssum")

            for s in range(ns):
                W = widths[s]
                off = offs[s]
                prod = ppool.tile([128, 32 * Wmax], bf16, tag="prod")
                for k in range(32):
                    nc.vector.tensor_scalar_mul(
                        prod[:, k * W : (k + 1) * W],
                        rxs[s][:, 0:W],
                        scmat[:, 32 * s + k : 32 * s + k + 1],
                    )
                expt = epool.tile([128, 32 * Wmax], bf16, tag="expt")
                # slot 0's first chunk is split small so ACT starts as soon
                # as the first two packs' products land
                bnds = [0, 2, 8, 16, 24, 32] if s == 0 else [0, 8, 16, 24, 32]
                for q in range(len(bnds) - 1):
                    nc.scalar.activation(
                        expt[:, bnds[q] * W : bnds[q + 1] * W],
                        prod[:, bnds[q] * W : bnds[q + 1] * W],
                        Exp,
                    )
                E_lo = psE.tile([64, Wmax], mybir.dt.float32, tag="Elo")
                E_hi = psE.tile([64, Wmax], mybir.dt.float32, tag="Ehi")
                for h in range(4):
                    E_t = E_lo if h < 2 else E_hi
                    rb = 32 * (h % 2)
                    for i in range(8):
                        k = 8 * h + i
                        nc.tensor.matmul(
                            E_t[rb : rb + 32, 0:W],
                            lhsT=onesbd[:, 32 * i : 32 * (i + 1)],
                            rhs=expt[:, k * W : (k + 1) * W],
                            start=(i == 0),
                            stop=(i == 7),
                        )
                nc.tensor.matmul(
                    Ssum[:, s : s + 1],
                    lhsT=fmat[0:32, lc + 128 * s : lc + 128 * (s + 1)],
                    rhs=fmat[0:32, yc + s : yc + s + 1],
                    start=True,
                    stop=True,
                )
                nc.scalar.activation(
                    logE[0:64, off : off + W], E_lo[:, 0:W], Ln
                )
                nc.scalar.activation(
                    logE[64:128, off : off + W], E_hi[:, 0:W], Ln
                )
                Lr = rpool.tile([128, 1], f32, tag="Lr")
                Lo = rpool.tile([128, 1], f32, tag="Lo")
                nc.vector.tensor_reduce(
                    Lr[:],
                    logE[:, off : off + W],
                    axis=mybir.AxisListType.X,
                    op=mybir.AluOpType.add,
                )
                nc.vector.tensor_reduce(
                    Lo[:],
                    logE[:, off : off + min(128, W)],
                    axis=mybir.AxisListType.X,
                    op=mybir.AluOpType.add,
                )
                mult = mybir.AluOpType.mult
                add = mybir.AluOpType.add
                nc.vector.scalar_tensor_tensor(
                    acc[:],
                    Ssum[:, s : s + 1],
                    fmat[:, s : s + 1],
                    acc[:],
                    mult,
                    add,
                )
                nc.vector.scalar_tensor_tensor(
                    acc[:],
                    Lr[:],
                    fmat[:, ns + s : ns + s + 1],
                    acc[:],
                    mult,
                    add,
                )
                nc.vector.scalar_tensor_tensor(
                    acc[:],
                    Lo[:],
                    fmat[:, 2 * ns + s : 2 * ns + s + 1],
                    acc[:],
                    mult,
                    add,
                )

            nc.sync.dma_start(out_d[:], acc[:])

    nc.compile()
    return nc


def _make_onesbd():
    onesbd = np.zeros((128, 8 * 32), ml_dtypes.bfloat16)
    for r in range(8):
        for a in range(4):
            onesbd[32 * a : 32 * (a + 1), 32 * r + 4 * r + a] = 1.0
    return onesbd


def _host_inputs(ps, widths, per_core, n):
    """Per-core input arrays for the compiled program."""
    ns = len(widths)
    TW = sum(widths)
    offs = [sum(widths[:s]) for s in range(ns)]
    onesbd = _make_onesbd()
    in_maps = []
    for c in range(N_CORES):
        rep4 = np.zeros((128, TW), np.float32)
        scal = np.zeros((128, ns * 32), np.float32)
        lhsa = np.zeros((32, ns * 128), np.float32)
        ysum = np.zeros((32, ns), np.float64)
        ws = np.zeros((128, ns), np.float32)
        wa = np.zeros((128, ns), np.float32)
        wb = np.zeros((128, ns), np.float32)
        kw = np.zeros((128, 1), np.float64)
        for s, slot in enumerate(per_core[c]):
            if slot is None:
                continue
            W = widths[s]
            off = offs[s]
            pos0, lo, hi, P = slot["pos0"], slot["lo"], slot["hi"], slot["P"]
            sc0, sl = slot["sfx"]
            # window real columns
            ywin = np.zeros((32, W), np.float32)
            yw = np.zeros(32, np.float64)
            nreal = sl
            ndo = 0
            if slot["own"] is not None:
                L1 = slot["own"][1]
                ocols = ps[pos0 + lo : pos0 + hi]  # [L1, 32]
                ywin[:, 0:L1] = ocols.T
                ywin[:, 128 : 128 + sl] = ps[sc0 : sc0 + sl].T
                yw = ocols.astype(np.float64).sum(axis=0) + 2.0 * ps[
                    sc0 : sc0 + sl
                ].astype(np.float64).sum(axis=0)
                nreal = L1 + sl
                ndo = 128 - L1
            else:
                ywin[:, 0:sl] = ps[sc0 : sc0 + sl].T
                yw = 2.0 * ps[sc0 : sc0 + sl].astype(np.float64).sum(axis=0)
            rep4[:, off : off + W] = np.tile(ywin, (4, 1))
            ablk = np.zeros((32, 128), np.float32)
            ablk[:, lo:hi] = ps[pos0 + lo : pos0 + hi].T
            lhsa[:, 128 * s : 128 * (s + 1)] = ablk
            scal[:, 32 * s : 32 * (s + 1)] = ablk.T.reshape(32, 128).T
            ysum[:, s] = yw
            wcol = -1.0 / (n * float(P) * float(P))
            ws[lo:hi, s] = wcol
            wa[:, s] = -2.0 * D * ws[:, s]
            if slot["own"] is not None:
                wb[:, s] = D * ws[:, s]
            ndf = W - nreal
            kw[:, 0] -= LN_D * (
                wa[:, s].astype(np.float64) * ndf
                + wb[:, s].astype(np.float64) * ndo
            )
        m = {}
        for s in range(ns):
            W = widths[s]
            blk = np.zeros((128, W + (256 if s == 0 else 0)), np.float32)
            blk[:, 0:W] = rep4[:, offs[s] : offs[s] + W]
            if s == 0:
                blk[:, W : W + 256] = onesbd.astype(np.float32)
            m[f"rx{s}"] = blk.astype(ml_dtypes.bfloat16)
        FC = 3 * ns + 1 + ns + 128 * ns
        fmat = np.zeros((128, FC), np.float32)
        fmat[:, 0:ns] = ws
        fmat[:, ns : 2 * ns] = wa
        fmat[:, 2 * ns : 3 * ns] = wb
        fmat[:, 3 * ns : 3 * ns + 1] = kw.astype(np.float32)
        yc = 3 * ns + 1
        lc = yc + ns
        fmat[0:32, yc : yc + ns] = ysum.astype(np.float32)
        fmat[0:32, lc : lc + 128 * ns] = lhsa
        m["fmat"] = fmat
        m["scmat"] = scal
        in_maps.append(m)
    return in_maps


def _prep(points, sensitive_attribute, t):
    points = np.asarray(points, dtype=np.float32)
    sa = np.asarray(sensitive_attribute).astype(np.int64)
    n, d = points.shape
    assert d == D
    scale = 1.0 / math.sqrt(float(np.asarray(t)))
    order = np.argsort(sa, kind="stable")
    ps = (points[order] * np.float32(scale)).astype(np.float32)
    widths, per_core = _plan(sa[order])
    return ps, widths, per_core, n


def simulate(points, sensitive_attribute, t):
    """Numpy emulation of the device program (for validation)."""
    ps, widths, per_core, n = _prep(points, sensitive_attribute, t)
    in_maps = _host_inputs(ps, widths, per_core, n)
    ns = len(widths)
    offs = [sum(widths[:s]) for s in range(ns)]
    total = 0.0
    for c in range(N_CORES):
        m = in_maps[c]
        fmat = m["fmat"]
        yc = 3 * ns + 1
        lc = yc + ns
        acc = fmat[:, 3 * ns].astype(np.float64).copy()
        for s in range(ns):
            W = widths[s]
            win = m[f"rx{s}"].astype(np.float32)[:, 0:W]
            scal = m["scmat"][:, 32 * s : 32 * (s + 1)]  # [128, 32] f32
            # prod[p, k, j] = win[p, j] * scal[p, k]
            prod = (win[:, None, :] * scal[:, :, None]).astype(
                ml_dtypes.bfloat16
            )
            ex = np.exp(prod.astype(np.float32)).astype(ml_dtypes.bfloat16)
            ex = ex.astype(np.float32)
            # E[a_hat=4k+a, j] = sum_d ex[32a+d, k, j]
            E = np.zeros((128, W), np.float32)
            for k in range(32):
                for a in range(4):
                    E[4 * k + a] = ex[32 * a : 32 * (a + 1), k, :].sum(axis=0)
            lnE = np.log(E)
            Lred = lnE.sum(axis=1)
            Lown = lnE[:, : min(128, W)].sum(axis=1)
            ablk = fmat[0:32, lc + 128 * s : lc + 128 * (s + 1)]
            Ssum = ablk.T @ fmat[0:32, yc + s]
            acc += (
                fmat[:, s].astype(np.float64) * Ssum
                + fmat[:, ns + s].astype(np.float64) * Lred
                + fmat[:, 2 * ns + s].astype(np.float64) * Lown
            )
        total += acc.sum()
    return np.float32(total)


def kernel(points, sensitive_attribute, t):
    _install_ntff_hook()
    _install_act_table_patch()

    ps, widths, per_core, n = _prep(points, sensitive_attribute, t)
    in_maps = _host_inputs(ps, widths, per_core, n)

    nc = _build_program(widths)
    trace = bool(int(os.environ.get("KERNEL_TRACE", "0")))
    res = run_bass_kernel_spmd(nc, in_maps, list(range(N_CORES)), trace=trace)
    last_run_info["exec_time_ns"] = res.exec_time_ns
    last_run_info["mean_exec_time_ns"] = res.mean_exec_time_ns
    last_run_info["W"] = widths
    last_run_info["ntiles"] = len(widths)
    last_run_info["instructions"] = (
        res.instructions_and_trace[0] if res.instructions_and_trace else None
    )

    total = 0.0
    for c in range(N_CORES):
        total += float(res.results[c]["out"].astype(np.float64).sum())
    return np.float32(total)
